# revision 26
# baseline (speedup 1.0000x reference)
"""DCGRU cell (nn_DCGRUCell) Trainium2 Bass kernel, 8 NeuronCores.

Sharding: node dimension N=4096 split 8 ways (512 rows/core); supports are
fed host-transposed (T = A^T), held resident in SBUF as bf16. Hop-1
diffusion products are computed node-major and AllGathered in two
column-chunks per GCN (A: bc cols 0-511; B: cols 512-1023 + 32 ragged) so
the gathers overlap hop-1/hop-2 compute. Hop-2 products are computed
directly in transposed (feature-major) form. All matmul operands are bf16
(PSUM fp32); the Chebyshev combination x2 = 2*A@x1 - x0 is folded into the
dense W matrices host-side. Moving operands are loaded 4 m-tiles per DMA;
the dense W stage processes batches 4 at a time with the candidate
transposes software-pipelined one iteration behind the matmuls. DMA issue
is split across the two HWDGE rings (sync=loads, scalar=stores).

kernel(**inputs) takes the FULL inputs from reference.setup_inputs() and
returns the FULL [16, 4096, 64] float32 output.
"""
import os
import numpy as np

import concourse.bass as bass
import concourse.mybir as mybir
import concourse.tile as tile
from concourse import bacc
from concourse.bass_utils import run_bass_kernel_spmd

F32 = mybir.dt.float32
BF16 = mybir.dt.bfloat16
AF = mybir.ActivationFunctionType

NCORES = 8
B, N, H, DIN = 16, 4096, 64, 2
C = DIN + H                 # 66 features per batch into each GCN
BC = B * C                  # 1056
NOWN = N // NCORES          # 512 rows per core
NT = NOWN // 128            # 4 n-tiles per core
MT = N // 128               # 32 m-tiles (contraction)
NQ = MT // NT               # 8 rank-blocks of 4 m-tiles
CA = 512                    # chunk A: bc columns 0:512
CB = 544                    # chunk B: bc columns 512:1024 + 32 ragged
MAIN = 1024
RAG = BC - MAIN             # 32 ragged columns
NB = 4                      # batches per W-stage iteration
GROUP = [list(range(NCORES))]

_NC_CACHE = {}


def build_nc():
    nc = bacc.Bacc("TRN2", target_bir_lowering=False, debug=False,
                   num_devices=NCORES)

    d = {}
    d["Ts"] = nc.dram_tensor("Ts", [2, N, NOWN], BF16, kind="ExternalInput")
    d["xs_main"] = nc.dram_tensor("xs_main", [N, MAIN], BF16,
                                  kind="ExternalInput")
    d["xs_rag"] = nc.dram_tensor("xs_rag", [MT, 128, RAG], BF16,
                                 kind="ExternalInput")
    d["xsT_own"] = nc.dram_tensor("xsT_own", [BC, NOWN], BF16,
                                  kind="ExternalInput")
    d["Wg"] = nc.dram_tensor("Wg", [5 * C, 2 * H], BF16, kind="ExternalInput")
    d["bg"] = nc.dram_tensor("bg", [2 * H, 1], F32, kind="ExternalInput")
    d["Wu"] = nc.dram_tensor("Wu", [5 * C, H], BF16, kind="ExternalInput")
    d["bu"] = nc.dram_tensor("bu", [H, 1], F32, kind="ExternalInput")
    d["outT"] = nc.dram_tensor("outT", [B, H, NOWN], F32,
                               kind="ExternalOutput")

    with tile.TileContext(nc) as tc:
        _emit(nc, tc, d)
    nc.compile()
    return nc


def _emit(nc, tc, d):
    import contextlib
    stack = contextlib.ExitStack()
    with stack:
        const = stack.enter_context(tc.tile_pool(name="const", bufs=1))
        sb_ex = stack.enter_context(tc.tile_pool(name="ex", bufs=1))
        sb_mov = stack.enter_context(tc.tile_pool(name="mov", bufs=1))
        sb_sm = stack.enter_context(tc.tile_pool(name="small", bufs=1))
        dram = stack.enter_context(
            tc.tile_pool(name="dram", bufs=1, space="DRAM"))
        psum = stack.enter_context(
            tc.tile_pool(name="psum", bufs=1, space="PSUM"))

        # ---- resident support tiles (loaded staggered in first sweep) ----
        Tch = {}
        for s in range(2):
            for k in range(NQ):
                Tch[(s, k)] = const.tile([128, NT, 512], BF16,
                                         name=f"T{s}_{k}")

        def load_Tch(k):
            for s in range(2):
                ts = d["Ts"].ap()[s].rearrange("(t p) n -> p t n", p=128)
                nc.scalar.dma_start(Tch[(s, k)][:],
                                    ts[:, k * NT:(k + 1) * NT, :])

        load_Tch(0)
        load_Tch(1)

        def T_tile(s, m):
            return Tch[(s, m // NT)][:, m % NT, :]

        ident = const.tile([128, 128], F32)
        nc.gpsimd.memset(ident[:], 0.0)
        nc.gpsimd.affine_select(
            out=ident[:], in_=ident[:],
            compare_op=mybir.AluOpType.not_equal, fill=1.0, base=0,
            pattern=[[-1, 128]], channel_multiplier=1)
        identb = const.tile([128, 128], BF16)
        nc.vector.tensor_copy(identb[:], ident[:])

        # dense-W constants: K-tiled [3, 110, out] (DMAs emitted later, just
        # before the gate loop, to keep the scalar ring clear at startup)
        KT = 110
        wg_t = const.tile([KT, 3, 2 * H], BF16)
        wu_t = const.tile([KT, 3, H], BF16)
        bg_t = const.tile([2 * H, 1], F32)
        bu_t = const.tile([H, 1], F32)

        def load_w_consts():
            for k in range(3):
                nc.scalar.dma_start(wg_t[:, k, :],
                                    d["Wg"].ap()[k * KT:(k + 1) * KT, :])
                nc.scalar.dma_start(wu_t[:, k, :],
                                    d["Wu"].ap()[k * KT:(k + 1) * KT, :])
            nc.scalar.dma_start(bg_t[:], d["bg"].ap())
            nc.scalar.dma_start(bu_t[:], d["bu"].ap())

        # row-run map: K-tile k's partition range [off, off+w) reads block
        # j (0 = direct input, 1-4 = diffusion outputs), feature cols
        # [c, c+w)
        KT_RUNS = []
        r0 = 0
        while r0 < 330:
            k, off = divmod(r0, KT)
            j, c = divmod(r0, C)
            w = min(C - c, KT - off)
            KT_RUNS.append((k, off, j, c, w))
            r0 += w

        # ---- DRAM staging ----
        agA = [dram.tile([2 * NT * 128 * CA], BF16, name=f"agA{g}")
               for g in range(2)]
        agB = [dram.tile([2 * NT * 128 * 512], BF16, name=f"agB{g}")
               for g in range(2)]
        agR = [dram.tile([2 * NT * 128 * RAG], BF16, name=f"agR{g}")
               for g in range(2)]
        agAo = [dram.tile([NCORES * 2 * NT * 128 * CA], BF16,
                          name=f"agAo{g}", addr_space="Shared")
                for g in range(2)]
        agBo = [dram.tile([NCORES * 2 * NT * 128 * 512], BF16,
                          name=f"agBo{g}", addr_space="Shared")
                for g in range(2)]
        agRo = [dram.tile([NCORES * 2 * NT * 128 * RAG], BF16,
                          name=f"agRo{g}", addr_space="Shared")
                for g in range(2)]
        candA = dram.tile([NT * 128 * CA], BF16, name="candA")
        candB = dram.tile([NT * 128 * 512], BF16, name="candB")
        candR = dram.tile([NT * 128 * RAG], BF16, name="candR")
        candAo = dram.tile([NCORES * NT * 128 * CA], BF16, name="candAo",
                           addr_space="Shared")
        candBo = dram.tile([NCORES * NT * 128 * 512], BF16, name="candBo",
                           addr_space="Shared")
        candRo = dram.tile([NCORES * NT * 128 * RAG], BF16, name="candRo",
                           addr_space="Shared")
        yt = [dram.tile([BC, NOWN], BF16, name=f"yt{i}") for i in range(4)]
        yt2 = [dram.tile([BC, NOWN], BF16, name=f"yt2_{i}") for i in range(4)]
        candT_dram = dram.tile([BC, NOWN], BF16)
        rt_dram = dram.tile([B, H, NOWN], BF16)

        def agA_own(g, s, t):
            o = ((s * NT + t) * 128) * CA
            return agA[g].opt()[o:o + 128 * CA].rearrange(
                "(p f) -> p f", f=CA)

        def agB_own(g, s, t):
            o = ((s * NT + t) * 128) * 512
            return agB[g].opt()[o:o + 128 * 512].rearrange(
                "(p f) -> p f", f=512)

        def agR_own(g, s, t):
            o = ((s * NT + t) * 128) * RAG
            return agR[g].opt()[o:o + 128 * RAG].rearrange(
                "(p f) -> p f", f=RAG)

        def outA_q(g, s, q):
            # rank q's 4 m-tiles for support s: [p, t, f]
            o = ((q * 2 + s) * NT * 128) * CA
            return agAo[g].opt()[o:o + NT * 128 * CA].rearrange(
                "(t p f) -> p t f", p=128, f=CA)

        def outB_q(g, s, q):
            o = ((q * 2 + s) * NT * 128) * 512
            return agBo[g].opt()[o:o + NT * 128 * 512].rearrange(
                "(t p f) -> p t f", p=128, f=512)

        def outR_s(g, s):
            # all ranks' rag blocks for support s: [p, q, t, f]
            v = agRo[g].opt().rearrange("(q s2 t p f) -> s2 p q t f",
                                        s2=2, t=NT, p=128, f=RAG)
            return v[s]

        def candA_own(t):
            o = t * 128 * CA
            return candA.opt()[o:o + 128 * CA].rearrange(
                "(p f) -> p f", f=CA)

        def candB_own(t):
            o = t * 128 * 512
            return candB.opt()[o:o + 128 * 512].rearrange(
                "(p f) -> p f", f=512)

        def candR_own(t):
            o = t * 128 * RAG
            return candR.opt()[o:o + 128 * RAG].rearrange(
                "(p f) -> p f", f=RAG)

        def candAo_q(q):
            o = q * NT * 128 * CA
            return candAo.opt()[o:o + NT * 128 * CA].rearrange(
                "(t p f) -> p t f", p=128, f=CA)

        def candBo_q(q):
            o = q * NT * 128 * 512
            return candBo.opt()[o:o + NT * 128 * 512].rearrange(
                "(t p f) -> p t f", p=128, f=512)

        def candRo_q(q):
            o = q * NT * 128 * RAG
            return candRo.opt()[o:o + NT * 128 * RAG].rearrange(
                "(t p f) -> p t f", p=128, f=RAG)

        def allgather(src, dst):
            nc.gpsimd.collective_compute(
                "AllGather", mybir.AluOpType.bypass, replica_groups=GROUP,
                ins=[src.opt()], outs=[dst.opt()])

        # ============ hop-1: node-major chunks + overlapped AG ============
        def emit_hop1_pair(pid, g, mov_main, mov_rag, yt_dst, stagger_T,
                           ld):
            """Y1_s[own rows, :] = A_s @ M for s in (0, 1); AG chunk A
            issued between the two main sweeps, chunk B after the ragged
            pass. Feature-major yt transposes are deferred (returned as a
            closure) so they land on the PE during the AG-B transfer.
            `ld` is the engine issuing moving-operand loads: for GCN1 the
            scalar ring (loads are AG-independent and must not sit behind
            hop-2's AG-gated loads on the sync ring); for GCN2 the sync
            ring (its loads are AG-gated like everything behind them)."""
            kept = {}
            preloaded = {}
            for hh in range(2):
                ps_m = {}
                for s in range(2):
                    for n in range(NT):
                        ps_m[(s, n)] = psum.tile(
                            [128, 512], F32, name=f"psm{pid}_{hh}{s}{n}",
                            tag="acc", bufs=8)
                for q in range(NQ):
                    if stagger_T and hh == 0 and q + 2 < NQ:
                        load_Tch(q + 2)
                    if (hh, q) in preloaded:
                        mv4 = preloaded.pop((hh, q))
                    else:
                        mv4 = sb_mov.tile([128, NT, 512], BF16,
                                          name=f"mv{pid}_{hh}_{q}",
                                          tag="mov", bufs=4)
                        ld.dma_start(mv4[:], mov_main(q, hh))
                    for tt in range(NT):
                        m = q * NT + tt
                        for s in range(2):
                            for n in range(NT):
                                nc.tensor.matmul(
                                    ps_m[(s, n)][:],
                                    T_tile(s, m)[:, n * 128:(n + 1) * 128],
                                    mv4[:, tt, :], start=(m == 0),
                                    stop=(m == MT - 1))
                if hh == 0:
                    # prefetch the next sweep's first tiles ahead of the
                    # staging stores so the ring never idles the PE at the
                    # sweep boundary
                    for qq in range(2):
                        mv4p = sb_mov.tile([128, NT, 512], BF16,
                                           name=f"mv{pid}_1_{qq}",
                                           tag="mov", bufs=4)
                        ld.dma_start(mv4p[:], mov_main(qq, 1))
                        preloaded[(1, qq)] = mv4p
                else:
                    # issue every ragged-pass load before the h1 staging
                    # stores: the loads must not queue up behind stores
                    # whose data is not ready yet (ring FIFO + shared
                    # completion-semaphore lanes both stall the rag MMs
                    # otherwise)
                    for q in range(NQ):
                        mvr4 = sb_mov.tile([128, NT, RAG], BF16,
                                           name=f"mvr{pid}_{q}",
                                           tag="movr", bufs=8)
                        ld.dma_start(mvr4[:], mov_rag(q))
                        preloaded[("r", q)] = mvr4
                for s in range(2):
                    exhs = []
                    for n in range(NT):
                        exh = sb_ex.tile([128, 512], BF16,
                                         name=f"ex{pid}{hh}{s}{n}",
                                         tag="ex", bufs=16)
                        nc.vector.tensor_copy(exh[:], ps_m[(s, n)][:])
                        dst = (agA_own(g, s, n) if hh == 0
                               else agB_own(g, s, n))
                        nc.scalar.dma_start(dst, exh[:])
                        exhs.append(exh)
                    kept[(hh, s)] = exhs
                allgather(*((agA[g], agAo[g]) if hh == 0
                            else (agB[g], agBo[g])))

            # ragged pass (node-major staging feeds chunk B)
            ps_t = [psum.tile([RAG, NOWN], F32, name=f"pst{pid}{s}",
                              tag="acc", bufs=8) for s in range(2)]
            for q in range(NQ):
                mvr4 = preloaded.pop(("r", q))
                for tt in range(NT):
                    m = q * NT + tt
                    for s in range(2):
                        nc.tensor.matmul(ps_t[s][:], mvr4[:, tt, :],
                                         T_tile(s, m), start=(m == 0),
                                         stop=(m == MT - 1))
            for s in range(2):
                rag_ex = sb_ex.tile([RAG, NOWN], BF16, name=f"rgex{pid}{s}",
                                    tag="ragex", bufs=2)
                nc.vector.tensor_copy(rag_ex[:], ps_t[s][:])
                nc.scalar.dma_start(yt_dst[s].opt()[MAIN:BC, :], rag_ex[:])
                for t in range(NT):
                    tp = psum.tile([128, RAG], BF16, name=f"rtp{pid}{s}",
                                   tag="acc", bufs=8)
                    nc.tensor.transpose(
                        tp[:], rag_ex[:, t * 128:(t + 1) * 128],
                        identb[0:RAG, 0:RAG])
                    rnm = sb_sm.tile([128, RAG], BF16, name=f"rnm{pid}{s}",
                                     tag="rnm", bufs=2)
                    nc.vector.tensor_copy(rnm[:], tp[:])
                    nc.scalar.dma_start(agR_own(g, s, t), rnm[:])
            allgather(agR[g], agRo[g])

            def deferred_yt():
                # two j-blocks per staging tile/store: halves the store
                # count so the transpose->copy->store chain stays ahead of
                # the PE
                for s in range(2):
                    for hh in range(2):
                        for j2 in range(2):
                            st4 = sb_sm.tile([128, 2, NOWN], BF16,
                                             name=f"st4{pid}", tag="st",
                                             bufs=4)
                            for jj2 in range(2):
                                j = j2 * 2 + jj2
                                for n in range(NT):
                                    tp = psum.tile([128, 128], BF16,
                                                   name=f"tp{pid}",
                                                   tag="acc", bufs=8)
                                    nc.tensor.transpose(
                                        tp[:],
                                        kept[(hh, s)][n][
                                            :, j * 128:(j + 1) * 128],
                                        identb[:])
                                    nc.vector.tensor_copy(
                                        st4[:, jj2,
                                            n * 128:(n + 1) * 128], tp[:])
                            jj = hh * 4 + j2 * 2
                            nc.scalar.dma_start(
                                yt_dst[s].opt()[jj * 128:(jj + 2) * 128, :]
                                .rearrange("(j p) n -> p j n", j=2),
                                st4[:])
            return deferred_yt

        # ======= hop-2 product: transposed form (feature-major out) =======
        def emit_hop2_pass(pid, s, g, part, yt_dst):
            """Y2raw^T[bc, own n] = (A_s @ Y1)^T for one column chunk."""
            ps = [psum.tile([128, NOWN], F32, name=f"ph2{pid}_{j}",
                            tag="acc", bufs=8) for j in range(4)]
            ps_r = None
            if part == 1:
                ps_r = psum.tile([RAG, NOWN], F32, name=f"ph2r{pid}",
                                 tag="acc", bufs=8)
                mrR = sb_mov.tile([128, NT, NQ, RAG], BF16,
                                  name=f"mrR{pid}", tag="mrR", bufs=2)
                vR = outR_s(g, s)
                for t in range(NT):
                    nc.sync.dma_start(mrR[:, t, :, :], vR[:, :, t, :])
            for q in range(NQ):
                mr4 = sb_mov.tile([128, NT, 512], BF16,
                                  name=f"mr{pid}_{q}", tag="mov", bufs=4)
                nc.sync.dma_start(
                    mr4[:], outA_q(g, s, q) if part == 0
                    else outB_q(g, s, q))
                for tt in range(NT):
                    m = q * NT + tt
                    for j in range(4):
                        nc.tensor.matmul(
                            ps[j][:],
                            mr4[:, tt, j * 128:(j + 1) * 128],
                            T_tile(s, m), start=(m == 0),
                            stop=(m == MT - 1))
                    if part == 1:
                        nc.tensor.matmul(ps_r[:], mrR[:, tt, q, :],
                                         T_tile(s, m), start=(m == 0),
                                         stop=(m == MT - 1))
            for j in range(4):
                exh = sb_ex.tile([128, NOWN], BF16, name=f"h2ex{pid}_{j}",
                                 tag="ex", bufs=16)
                nc.vector.tensor_copy(exh[:], ps[j][:])
                jj = j if part == 0 else 4 + j
                nc.scalar.dma_start(
                    yt_dst.opt()[jj * 128:(jj + 1) * 128, :], exh[:])
            if part == 1:
                exr = sb_ex.tile([RAG, NOWN], BF16, name=f"h2exr{pid}",
                                 tag="ragex", bufs=2)
                nc.vector.tensor_copy(exr[:], ps_r[:])
                nc.scalar.dma_start(yt_dst.opt()[MAIN:BC, :], exr[:])

        # ======================= GCN 1 (gate) =======================
        def g1_main(q, hh):
            src = d["xs_main"].ap().rearrange("(q t p) f -> q p t f",
                                              p=128, t=NT)
            return src[q, :, :, hh * 512:(hh + 1) * 512]

        def g1_rag(q):
            src = d["xs_rag"].ap().rearrange("(q t) p f -> q p t f", t=NT)
            return src[q]

        dyt1 = emit_hop1_pair("g1h1", 0, g1_main, g1_rag,
                              (yt[0], yt[2]), True, nc.scalar)
        dyt1()
        emit_hop2_pass("g1s0h2A", 0, 0, 0, yt[1])
        emit_hop2_pass("g1s1h2A", 1, 0, 0, yt[3])
        emit_hop2_pass("g1s0h2B", 0, 0, 1, yt[1])
        emit_hop2_pass("g1s1h2B", 1, 0, 1, yt[3])

        # ----- gate W-stage + candidate build, 4 batches per iter; the
        # candidate transpose/staging runs one iteration behind so the PE
        # queue alternates matmul-block, transpose-block without stalling.
        def load_ktiles(pi, block0_src, ysrc):
            """Assemble the 330-feature contraction as 3 dense K-tiles of
            110 partitions; block 0 comes from block0_src, blocks 1-4 from
            ysrc[j-1] (the staged diffusion outputs)."""
            b0 = NB * pi
            kts = [sb_sm.tile([KT, NB, NOWN], BF16, name=f"kt{k}",
                              tag=f"kt{k}", bufs=3) for k in range(3)]
            for k, off, j, c, w in KT_RUNS:
                src = block0_src if j == 0 else ysrc[j - 1].opt()
                nc.sync.dma_start(
                    kts[k][off:off + w, :, :],
                    src[b0 * C:(b0 + NB) * C, :]
                    .rearrange("(b c) n -> c b n", b=NB)[c:c + w])
            return kts

        def gate_mm(pi):
            b0 = NB * pi
            kts = load_ktiles(pi, d["xsT_own"].ap(), yt)
            zr_ps = [psum.tile([2 * H, NOWN], F32, name=f"zrps{b2}",
                               tag="acc", bufs=8) for b2 in range(NB)]
            for k in range(3):
                for b2 in range(NB):
                    nc.tensor.matmul(zr_ps[b2][:], wg_t[:, k, :],
                                     kts[k][:, b2, :],
                                     start=(k == 0), stop=(k == 2))
            zr4 = sb_sm.tile([2 * H, NB, NOWN], BF16, name="zr", tag="zr",
                             bufs=2)
            for b2 in range(NB):
                nc.scalar.activation(zr4[:, b2, :], zr_ps[b2][:],
                                     AF.Sigmoid, bias=bg_t[:])
            nc.scalar.dma_start(
                rt_dram.opt()[b0:b0 + NB].rearrange("b (h n) -> h b n",
                                                    h=H),
                zr4[H:2 * H, :, :])
            # kts[0][0:C] holds the direct input rows [state(64); x(2)]
            cT4 = sb_sm.tile([C, NB, NOWN], BF16, name="cT", tag="cT",
                             bufs=2)
            nc.vector.tensor_mul(cT4[0:H, :, :], zr4[0:H, :, :],
                                 kts[0][0:H, :, :])
            nc.vector.tensor_copy(cT4[H:C, :, :], kts[0][H:C, :, :])
            nc.scalar.dma_start(
                candT_dram.opt()[b0 * C:(b0 + NB) * C, :]
                .rearrange("(b c) n -> c b n", b=NB), cT4[:])
            return cT4

        def cand_stage(pi, cT4):
            c0 = NB * pi * C
            hi = c0 + NB * C
            for t in range(NT):
                ct4 = sb_sm.tile([128, NB, C], BF16, name="ctnm",
                                 tag="ctnm", bufs=2)
                for b2 in range(NB):
                    tp = psum.tile([128, C], BF16, name="ctps", tag="acc",
                                   bufs=8)
                    nc.tensor.transpose(
                        tp[:], cT4[:, b2, t * 128:(t + 1) * 128],
                        identb[0:C, 0:C])
                    nc.vector.tensor_copy(ct4[:, b2, :], tp[:])
                flat = ct4[:].rearrange("p b c -> p (b c)")
                segs = []
                if c0 < CA:
                    e = min(hi, CA)
                    segs.append((candA_own(t)[:, c0:e], 0, e - c0))
                if hi > CA and c0 < MAIN:
                    s0 = max(c0, CA)
                    e = min(hi, MAIN)
                    segs.append((candB_own(t)[:, s0 - CA:e - CA],
                                 s0 - c0, e - s0))
                if hi > MAIN:
                    s0 = max(c0, MAIN)
                    segs.append((candR_own(t)[:, s0 - MAIN:hi - MAIN],
                                 s0 - c0, hi - s0))
                for dst, off, w in segs:
                    nc.scalar.dma_start(dst, flat[:, off:off + w])

        load_w_consts()
        cts = [gate_mm(0), gate_mm(1)]
        cand_stage(0, cts[0])
        cand_stage(1, cts[1])
        allgather(candA, candAo)
        cts.append(gate_mm(2))
        cts.append(gate_mm(3))
        cand_stage(2, cts[2])
        cand_stage(3, cts[3])
        allgather(candB, candBo)
        allgather(candR, candRo)

        # ======================= GCN 2 (update) =======================
        def g2_main(q, hh):
            if hh == 0:
                return candAo_q(q)
            return candBo_q(q)

        def g2_rag(q):
            return candRo_q(q)

        dyt2 = emit_hop1_pair("g2h1", 1, g2_main, g2_rag,
                              (yt2[0], yt2[2]), False, nc.sync)
        dyt2()
        emit_hop2_pass("g2s0h2A", 0, 1, 0, yt2[1])
        emit_hop2_pass("g2s1h2A", 1, 1, 0, yt2[3])
        emit_hop2_pass("g2s0h2B", 0, 1, 1, yt2[1])
        emit_hop2_pass("g2s1h2B", 1, 1, 1, yt2[3])

        # update W-stage + final combine, 4 batches per iter
        for pi in range(B // NB):
            b0 = NB * pi
            kts = load_ktiles(pi, candT_dram.opt(), yt2)
            hc_ps = [psum.tile([H, NOWN], F32, name=f"hcps{b2}", tag="acc",
                               bufs=8) for b2 in range(NB)]
            for k in range(3):
                for b2 in range(NB):
                    nc.tensor.matmul(hc_ps[b2][:], wu_t[:, k, :],
                                     kts[k][:, b2, :],
                                     start=(k == 0), stop=(k == 2))
            hc4 = sb_sm.tile([H, NB, NOWN], BF16, name="hc", tag="zr",
                             bufs=2)
            for b2 in range(NB):
                nc.scalar.activation(hc4[:, b2, :], hc_ps[b2][:], AF.Tanh,
                                     bias=bu_t[:])

            # out = hc + r * (state - hc); state rows are xsT_own[0:H]
            stT4 = sb_sm.tile([H, NB, NOWN], BF16, name="stTu", tag="stg",
                              bufs=2)
            nc.sync.dma_start(
                stT4[:],
                d["xsT_own"].ap()[b0 * C:(b0 + NB) * C, :]
                .rearrange("(b c) n -> c b n", b=NB)[0:H])
            rT4 = sb_sm.tile([H, NB, NOWN], BF16, name="rT", tag="rT",
                             bufs=2)
            nc.sync.dma_start(
                rT4[:],
                rt_dram.opt()[b0:b0 + NB].rearrange("b (h n) -> h b n",
                                                    h=H))
            tmp4 = sb_sm.tile([H, NB, NOWN], BF16, name="tmp", tag="tmp",
                              bufs=2)
            nc.vector.tensor_sub(tmp4[:], stT4[:], hc4[:])
            nc.vector.tensor_mul(tmp4[:], rT4[:], tmp4[:])
            ot4 = sb_sm.tile([H, NB, NOWN], F32, name="ot", tag="ot",
                             bufs=1)
            nc.vector.tensor_add(ot4[:], hc4[:], tmp4[:])
            nc.scalar.dma_start(
                d["outT"].ap()[b0:b0 + NB].rearrange("b h n -> h b n"),
                ot4[:])


def prepare_in_maps(x, state, support0, support1, W_gate, b_gate,
                    W_update, b_update):
    BFNP = mybir.dt.np(BF16)
    xs = np.concatenate([x, state], axis=-1)          # [B, N, C]
    xs_nm = np.ascontiguousarray(
        xs.transpose(1, 0, 2).reshape(N, BC)).astype(BFNP)
    # feature-major input for W / elementwise uses [state(64); x(2)] rows
    sx_nm = np.ascontiguousarray(
        np.concatenate([state, x], axis=-1)
        .transpose(1, 0, 2).reshape(N, BC)).astype(np.float32)
    perm = np.r_[DIN:C, 0:DIN]                 # [x, state] -> [state, x]

    # fold the Chebyshev combination x2 = 2*A@x1 - x0 into W:
    # W0 -= (W2 + W4); W2 *= 2; W4 *= 2  (per 66-row block)
    def fold(W):
        Wf = np.ascontiguousarray(W, dtype=np.float32).copy()
        Wf[0:C] -= Wf[2 * C:3 * C] + Wf[4 * C:5 * C]
        Wf[2 * C:3 * C] *= 2.0
        Wf[4 * C:5 * C] *= 2.0
        return Wf

    Wg_dev = fold(W_gate)
    Wg_dev[0:C] = Wg_dev[0:C][perm]            # only the X-block reads xsT
    Wu_dev = fold(W_update)
    for j in range(5):                         # all of cand's blocks permute
        Wu_dev[j * C:(j + 1) * C] = Wu_dev[j * C:(j + 1) * C][perm]
    Wg_dev = Wg_dev.astype(BFNP)
    Wu_dev = Wu_dev.astype(BFNP)

    xs_main = np.ascontiguousarray(xs_nm[:, :MAIN])
    xs_rag = np.ascontiguousarray(xs_nm[:, MAIN:]).reshape(MT, 128, RAG)
    bg = np.ascontiguousarray(b_gate, dtype=np.float32).reshape(2 * H, 1)
    bu = np.ascontiguousarray(b_update, dtype=np.float32).reshape(H, 1)
    s0b = np.asarray(support0, dtype=np.float32).astype(BFNP)
    s1b = np.asarray(support1, dtype=np.float32).astype(BFNP)

    in_maps = []
    for r in range(NCORES):
        n0 = r * NOWN
        in_maps.append({
            "Ts": np.ascontiguousarray(
                np.stack([s0b[n0:n0 + NOWN, :].T,
                          s1b[n0:n0 + NOWN, :].T])),
            "xs_main": xs_main,
            "xs_rag": xs_rag,
            "xsT_own": np.ascontiguousarray(
                sx_nm[n0:n0 + NOWN].T).astype(BFNP),
            "Wg": Wg_dev, "bg": bg, "Wu": Wu_dev, "bu": bu,
        })
    return in_maps


def assemble_output(results):
    out = np.empty((B, N, H), dtype=np.float32)
    for r in range(NCORES):
        n0 = r * NOWN
        out[:, n0:n0 + NOWN, :] = results[r]["outT"].transpose(0, 2, 1)
    return out


def get_nc():
    if "nc" not in _NC_CACHE:
        _NC_CACHE["nc"] = build_nc()
    return _NC_CACHE["nc"]


def kernel(x, state, support0, support1, W_gate, b_gate, W_update, b_update):
    nc = get_nc()
    in_maps = prepare_in_maps(x, state, support0, support1,
                              W_gate, b_gate, W_update, b_update)
    prev = os.environ.get("BASS_NEVER_TRACE")
    os.environ["BASS_NEVER_TRACE"] = "1"
    try:
        res = run_bass_kernel_spmd(nc, in_maps, list(range(NCORES)),
                                   trace=False)
    finally:
        if prev is None:
            os.environ.pop("BASS_NEVER_TRACE", None)
        else:
            os.environ["BASS_NEVER_TRACE"] = prev
    return assemble_output(res.results)


# revision 27
# speedup vs baseline: 1.0217x; 1.0217x over previous
"""DCGRU cell (nn_DCGRUCell) Trainium2 Bass kernel, 8 NeuronCores.

Sharding: node dimension N=4096 split 8 ways (512 rows/core); supports are
fed host-transposed (T = A^T), held resident in SBUF as bf16. Hop-1
diffusion products are computed node-major and AllGathered in three
column-chunks per GCN (A: bc cols 0-511 after the first sweep; B: cols
512-1023 after the second; R: 32 ragged) so the gathers overlap hop-1 and
hop-2 compute. Hop-2 products are computed directly in transposed
(feature-major) form. All matmul operands are bf16 (PSUM fp32); the
Chebyshev combination x2 = 2*A@x1 - x0 is folded into the dense W matrices
host-side, and the 330-feature dense contraction runs as 3 dense K-tiles
of 110. The dense W stage processes batches 4 at a time, interleaved with
the hop-2 passes (iteration 0 runs between the A and B passes), and the
candidate is gathered in four batch-aligned chunks, each fired as soon as
its batches are staged. DMA issue is split across the two HWDGE rings
(scalar ring: GCN1/support loads + stores; sync ring: AG-gated loads).

kernel(**inputs) takes the FULL inputs from reference.setup_inputs() and
returns the FULL [16, 4096, 64] float32 output.
"""
import os
import numpy as np

import concourse.bass as bass
import concourse.mybir as mybir
import concourse.tile as tile
from concourse import bacc
from concourse.bass_utils import run_bass_kernel_spmd

F32 = mybir.dt.float32
BF16 = mybir.dt.bfloat16
AF = mybir.ActivationFunctionType

NCORES = 8
B, N, H, DIN = 16, 4096, 64, 2
C = DIN + H                 # 66 features per batch into each GCN
BC = B * C                  # 1056
NOWN = N // NCORES          # 512 rows per core
NT = NOWN // 128            # 4 n-tiles per core
MT = N // 128               # 32 m-tiles (contraction)
NQ = MT // NT               # 8 rank-blocks of 4 m-tiles
MAIN = 1024
RAG = BC - MAIN             # 32 ragged columns
NB = 4                      # batches per W-stage iteration
CC4 = NB * C                # 264 bc columns per candidate chunk
KT = 110                    # dense-W K-tile height (330 = 3*110)
GROUP = [list(range(NCORES))]

_NC_CACHE = {}


def build_nc():
    nc = bacc.Bacc("TRN2", target_bir_lowering=False, debug=False,
                   num_devices=NCORES)

    d = {}
    d["Ts"] = nc.dram_tensor("Ts", [2, N, NOWN], BF16, kind="ExternalInput")
    d["xs_main"] = nc.dram_tensor("xs_main", [N, MAIN], BF16,
                                  kind="ExternalInput")
    d["xs_rag"] = nc.dram_tensor("xs_rag", [MT, 128, RAG], BF16,
                                 kind="ExternalInput")
    d["xsT_own"] = nc.dram_tensor("xsT_own", [BC, NOWN], BF16,
                                  kind="ExternalInput")
    d["Wg"] = nc.dram_tensor("Wg", [5 * C, 2 * H], BF16, kind="ExternalInput")
    d["bg"] = nc.dram_tensor("bg", [2 * H, 1], F32, kind="ExternalInput")
    d["Wu"] = nc.dram_tensor("Wu", [5 * C, H], BF16, kind="ExternalInput")
    d["bu"] = nc.dram_tensor("bu", [H, 1], F32, kind="ExternalInput")
    d["outT"] = nc.dram_tensor("outT", [B, H, NOWN], F32,
                               kind="ExternalOutput")

    with tile.TileContext(nc) as tc:
        _emit(nc, tc, d)
    nc.compile()
    return nc


def _emit(nc, tc, d):
    import contextlib
    stack = contextlib.ExitStack()
    with stack:
        const = stack.enter_context(tc.tile_pool(name="const", bufs=1))
        sb_ex = stack.enter_context(tc.tile_pool(name="ex", bufs=1))
        sb_mov = stack.enter_context(tc.tile_pool(name="mov", bufs=1))
        sb_sm = stack.enter_context(tc.tile_pool(name="small", bufs=1))
        dram = stack.enter_context(
            tc.tile_pool(name="dram", bufs=1, space="DRAM"))
        psum = stack.enter_context(
            tc.tile_pool(name="psum", bufs=1, space="PSUM"))

        # ---- resident support tiles (loaded staggered in first sweep) ----
        Tch = {}
        for s in range(2):
            for k in range(NQ):
                Tch[(s, k)] = const.tile([128, NT, 512], BF16,
                                         name=f"T{s}_{k}")

        def load_Tch(k):
            for s in range(2):
                ts = d["Ts"].ap()[s].rearrange("(t p) n -> p t n", p=128)
                nc.scalar.dma_start(Tch[(s, k)][:],
                                    ts[:, k * NT:(k + 1) * NT, :])

        load_Tch(0)
        load_Tch(1)

        def T_tile(s, m):
            return Tch[(s, m // NT)][:, m % NT, :]

        ident = const.tile([128, 128], F32)
        nc.gpsimd.memset(ident[:], 0.0)
        nc.gpsimd.affine_select(
            out=ident[:], in_=ident[:],
            compare_op=mybir.AluOpType.not_equal, fill=1.0, base=0,
            pattern=[[-1, 128]], channel_multiplier=1)
        identb = const.tile([128, 128], BF16)
        nc.vector.tensor_copy(identb[:], ident[:])

        # dense-W constants: K-tiled [3, 110, out] (DMAs emitted later so
        # the scalar ring stays clear at startup)
        wg_t = const.tile([KT, 3, 2 * H], BF16)
        wu_t = const.tile([KT, 3, H], BF16)
        bg_t = const.tile([2 * H, 1], F32)
        bu_t = const.tile([H, 1], F32)

        def load_w_consts():
            for k in range(3):
                nc.scalar.dma_start(wg_t[:, k, :],
                                    d["Wg"].ap()[k * KT:(k + 1) * KT, :])
                nc.scalar.dma_start(wu_t[:, k, :],
                                    d["Wu"].ap()[k * KT:(k + 1) * KT, :])
            nc.scalar.dma_start(bg_t[:], d["bg"].ap())
            nc.scalar.dma_start(bu_t[:], d["bu"].ap())

        # row-run map for the dense stage: K-tile k's partition range
        # [off, off+w) reads block j (0 = direct input, 1-4 = diffusion
        # outputs y1_s0, y2_s0, y1_s1, y2_s1), feature cols [c, c+w)
        KT_RUNS = []
        r0 = 0
        while r0 < 330:
            k, off = divmod(r0, KT)
            j, c = divmod(r0, C)
            w = min(C - c, KT - off)
            KT_RUNS.append((k, off, j, c, w))
            r0 += w

        # ---- DRAM staging ----
        # per GCN g: node-major y1 chunks A (cols 0:512), B (512:1024),
        # R (32 ragged) + their gathers
        agA = [dram.tile([2 * NT * 128 * 512], BF16, name=f"agA{g}")
               for g in range(2)]
        agB = [dram.tile([2 * NT * 128 * 512], BF16, name=f"agB{g}")
               for g in range(2)]
        agR = [dram.tile([2 * NT * 128 * RAG], BF16, name=f"agR{g}")
               for g in range(2)]
        agAo = [dram.tile([NCORES * 2 * NT * 128 * 512], BF16,
                          name=f"agAo{g}", addr_space="Shared")
                for g in range(2)]
        agBo = [dram.tile([NCORES * 2 * NT * 128 * 512], BF16,
                          name=f"agBo{g}", addr_space="Shared")
                for g in range(2)]
        agRo = [dram.tile([NCORES * 2 * NT * 128 * RAG], BF16,
                          name=f"agRo{g}", addr_space="Shared")
                for g in range(2)]
        # candidate: four batch-aligned node-major chunks of 264 bc cols
        candC = [dram.tile([NT * 128 * CC4], BF16, name=f"candC{i}")
                 for i in range(4)]
        candCo = [dram.tile([NCORES * NT * 128 * CC4], BF16,
                            name=f"candCo{i}", addr_space="Shared")
                  for i in range(4)]
        # feature-major staging: y1 whole, y2 split at bc row 512 (A pad
        # to 528 so 4-batch reads can slice past the written 512 rows)
        y1t = [[dram.tile([BC, NOWN], BF16, name=f"y1_{g}{s}")
                for s in range(2)] for g in range(2)]
        y2tA = [[dram.tile([528, NOWN], BF16, name=f"y2A_{g}{s}")
                 for s in range(2)] for g in range(2)]
        y2tB = [[dram.tile([544, NOWN], BF16, name=f"y2B_{g}{s}")
                 for s in range(2)] for g in range(2)]
        candT_dram = dram.tile([BC, NOWN], BF16)
        rt_dram = dram.tile([B, H, NOWN], BF16)

        def agA_own(g, s, t):
            o = ((s * NT + t) * 128) * 512
            return agA[g].opt()[o:o + 128 * 512].rearrange(
                "(p f) -> p f", f=512)

        def agB_own(g, s, t):
            o = ((s * NT + t) * 128) * 512
            return agB[g].opt()[o:o + 128 * 512].rearrange(
                "(p f) -> p f", f=512)

        def agR_own(g, s, t):
            o = ((s * NT + t) * 128) * RAG
            return agR[g].opt()[o:o + 128 * RAG].rearrange(
                "(p f) -> p f", f=RAG)

        def outA_q(g, s, q):
            o = ((q * 2 + s) * NT * 128) * 512
            return agAo[g].opt()[o:o + NT * 128 * 512].rearrange(
                "(t p f) -> p t f", p=128, f=512)

        def outB_q(g, s, q):
            o = ((q * 2 + s) * NT * 128) * 512
            return agBo[g].opt()[o:o + NT * 128 * 512].rearrange(
                "(t p f) -> p t f", p=128, f=512)

        def outR_s(g, s):
            v = agRo[g].opt().rearrange("(q s2 t p f) -> s2 p q t f",
                                        s2=2, t=NT, p=128, f=RAG)
            return v[s]

        def candC_own(i, t):
            o = t * 128 * CC4
            return candC[i].opt()[o:o + 128 * CC4].rearrange(
                "(p f) -> p f", f=CC4)

        def candCo_q(i, q):
            o = q * NT * 128 * CC4
            return candCo[i].opt()[o:o + NT * 128 * CC4].rearrange(
                "(t p f) -> p t f", p=128, f=CC4)

        def allgather(src, dst):
            nc.gpsimd.collective_compute(
                "AllGather", mybir.AluOpType.bypass, replica_groups=GROUP,
                ins=[src.opt()], outs=[dst.opt()])

        # ============ hop-1: node-major chunks + overlapped AG ============
        def emit_hop1_pair(pid, g, load_mov, load_rag, y1_dst, stagger_T):
            """Y1_s[own rows, :] = A_s @ M for s in (0, 1). AG chunk A
            fires after the first main sweep, chunk B after the second,
            chunk R after the ragged pass. Feature-major y1 transposes are
            deferred (returned as a closure) so they land on the PE during
            the AG transfers. load_mov/load_rag issue the moving-operand
            DMAs (GCN1: scalar ring, AG-independent; GCN2: sync ring,
            AG-gated like everything queued behind them)."""
            kept = {}
            preloaded = {}
            for hh in range(2):
                ps_m = {}
                for s in range(2):
                    for n in range(NT):
                        ps_m[(s, n)] = psum.tile(
                            [128, 512], F32, name=f"psm{pid}_{hh}{s}{n}",
                            tag="acc", bufs=8)
                for q in range(NQ):
                    if stagger_T and hh == 0 and q + 2 < NQ:
                        load_Tch(q + 2)
                    if (hh, q) in preloaded:
                        mv4 = preloaded.pop((hh, q))
                    else:
                        mv4 = sb_mov.tile([128, NT, 512], BF16,
                                          name=f"mv{pid}_{hh}_{q}",
                                          tag="mov", bufs=4)
                        load_mov(mv4, q, hh)
                    for tt in range(NT):
                        m = q * NT + tt
                        for s in range(2):
                            for n in range(NT):
                                nc.tensor.matmul(
                                    ps_m[(s, n)][:],
                                    T_tile(s, m)[:, n * 128:(n + 1) * 128],
                                    mv4[:, tt, :], start=(m == 0),
                                    stop=(m == MT - 1))
                if hh == 0:
                    # prefetch the next sweep's first tiles ahead of the
                    # staging stores so the ring never idles the PE at the
                    # sweep boundary
                    for qq in range(2):
                        mv4p = sb_mov.tile([128, NT, 512], BF16,
                                           name=f"mv{pid}_1_{qq}",
                                           tag="mov", bufs=4)
                        load_mov(mv4p, qq, 1)
                        preloaded[(1, qq)] = mv4p
                else:
                    # issue every ragged-pass load before the h1 staging
                    # stores (ring FIFO + shared completion-semaphore
                    # lanes would otherwise stall the rag MMs behind
                    # stores whose data is not ready)
                    for q in range(NQ):
                        mvr4 = sb_mov.tile([128, NT, RAG], BF16,
                                           name=f"mvr{pid}_{q}",
                                           tag="movr", bufs=8)
                        load_rag(mvr4, q)
                        preloaded[("r", q)] = mvr4
                for s in range(2):
                    exhs = []
                    for n in range(NT):
                        exh = sb_ex.tile([128, 512], BF16,
                                         name=f"ex{pid}{hh}{s}{n}",
                                         tag="ex", bufs=16)
                        nc.vector.tensor_copy(exh[:], ps_m[(s, n)][:])
                        dst = (agA_own(g, s, n) if hh == 0
                               else agB_own(g, s, n))
                        nc.scalar.dma_start(dst, exh[:])
                        exhs.append(exh)
                    kept[(hh, s)] = exhs
                allgather(*((agA[g], agAo[g]) if hh == 0
                            else (agB[g], agBo[g])))

            # ragged pass (node-major staging feeds chunk R)
            ps_t = [psum.tile([RAG, NOWN], F32, name=f"pst{pid}{s}",
                              tag="acc", bufs=8) for s in range(2)]
            for q in range(NQ):
                mvr4 = preloaded.pop(("r", q))
                for tt in range(NT):
                    m = q * NT + tt
                    for s in range(2):
                        nc.tensor.matmul(ps_t[s][:], mvr4[:, tt, :],
                                         T_tile(s, m), start=(m == 0),
                                         stop=(m == MT - 1))
            for s in range(2):
                rag_ex = sb_ex.tile([RAG, NOWN], BF16, name=f"rgex{pid}{s}",
                                    tag="ragex", bufs=2)
                nc.vector.tensor_copy(rag_ex[:], ps_t[s][:])
                nc.scalar.dma_start(y1_dst[s].opt()[MAIN:BC, :], rag_ex[:])
                for t in range(NT):
                    tp = psum.tile([128, RAG], BF16, name=f"rtp{pid}{s}",
                                   tag="acc", bufs=8)
                    nc.tensor.transpose(
                        tp[:], rag_ex[:, t * 128:(t + 1) * 128],
                        identb[0:RAG, 0:RAG])
                    rnm = sb_sm.tile([128, RAG], BF16, name=f"rnm{pid}{s}",
                                     tag="rnm", bufs=2)
                    nc.vector.tensor_copy(rnm[:], tp[:])
                    nc.scalar.dma_start(agR_own(g, s, t), rnm[:])
            allgather(agR[g], agRo[g])

            def deferred_yt():
                # two j-blocks per staging tile/store: halves the store
                # count so the transpose->copy->store chain stays ahead of
                # the PE
                for s in range(2):
                    for hh in range(2):
                        for j2 in range(2):
                            st4 = sb_sm.tile([128, 2, NOWN], BF16,
                                             name=f"st4{pid}", tag="st",
                                             bufs=4)
                            for jj2 in range(2):
                                j = j2 * 2 + jj2
                                for n in range(NT):
                                    tp = psum.tile([128, 128], BF16,
                                                   name=f"tp{pid}",
                                                   tag="acc", bufs=8)
                                    nc.tensor.transpose(
                                        tp[:],
                                        kept[(hh, s)][n][
                                            :, j * 128:(j + 1) * 128],
                                        identb[:])
                                    nc.vector.tensor_copy(
                                        st4[:, jj2,
                                            n * 128:(n + 1) * 128], tp[:])
                            jj = hh * 4 + j2 * 2
                            nc.scalar.dma_start(
                                y1_dst[s].opt()[jj * 128:(jj + 2) * 128, :]
                                .rearrange("(j p) n -> p j n", j=2),
                                st4[:])
            return deferred_yt

        # ======= hop-2 product: transposed form (feature-major out) =======
        def emit_hop2_pass(pid, s, g, part, yA_dst, yB_dst):
            """Y2raw^T[bc, own n] = (A_s @ Y1)^T for one column chunk."""
            ps = [psum.tile([128, NOWN], F32, name=f"ph2{pid}_{j}",
                            tag="acc", bufs=8) for j in range(4)]
            ps_r = None
            if part == 1:
                ps_r = psum.tile([RAG, NOWN], F32, name=f"ph2r{pid}",
                                 tag="acc", bufs=8)
                mrR = sb_mov.tile([128, NT, NQ, RAG], BF16,
                                  name=f"mrR{pid}", tag="mrR", bufs=2)
                vR = outR_s(g, s)
                for t in range(NT):
                    nc.sync.dma_start(mrR[:, t, :, :], vR[:, :, t, :])
            for q in range(NQ):
                mr4 = sb_mov.tile([128, NT, 512], BF16,
                                  name=f"mr{pid}_{q}", tag="mov", bufs=4)
                nc.sync.dma_start(
                    mr4[:], outA_q(g, s, q) if part == 0
                    else outB_q(g, s, q))
                for tt in range(NT):
                    m = q * NT + tt
                    for j in range(4):
                        nc.tensor.matmul(
                            ps[j][:],
                            mr4[:, tt, j * 128:(j + 1) * 128],
                            T_tile(s, m), start=(m == 0),
                            stop=(m == MT - 1))
                    if part == 1:
                        nc.tensor.matmul(ps_r[:], mrR[:, tt, q, :],
                                         T_tile(s, m), start=(m == 0),
                                         stop=(m == MT - 1))
            dst_t = yA_dst if part == 0 else yB_dst
            for j in range(4):
                exh = sb_ex.tile([128, NOWN], BF16, name=f"h2ex{pid}_{j}",
                                 tag="ex", bufs=16)
                nc.vector.tensor_copy(exh[:], ps[j][:])
                nc.scalar.dma_start(
                    dst_t.opt()[j * 128:(j + 1) * 128, :], exh[:])
            if part == 1:
                exr = sb_ex.tile([RAG, NOWN], BF16, name=f"h2exr{pid}",
                                 tag="ragex", bufs=2)
                nc.vector.tensor_copy(exr[:], ps_r[:])
                nc.scalar.dma_start(yB_dst.opt()[512:544, :], exr[:])

        # ============== dense W stage (K-tiled, 4 batches/iter) ==========
        def load_ktiles(pi, block0_src, g):
            """Assemble the 330-feature contraction as 3 dense K-tiles of
            110 partitions. y2 rows split across the A (rows < 512, padded
            tile) and B (rows 512+, incl ragged) staging tiles; batch 7
            straddles the boundary."""
            b0 = NB * pi
            kts = [sb_sm.tile([KT, NB, NOWN], BF16, name=f"kt{k}",
                              tag=f"kt{k}", bufs=3) for k in range(3)]
            for k, off, j, c, w in KT_RUNS:
                if j == 0 or j % 2 == 1:
                    src = block0_src if j == 0 else y1t[g][(j - 1) // 2].opt()
                    nc.sync.dma_start(
                        kts[k][off:off + w, :, :],
                        src[b0 * C:(b0 + NB) * C, :]
                        .rearrange("(b c) n -> c b n", b=NB)[c:c + w])
                    continue
                sidx = j // 2 - 1
                tA, tB = y2tA[g][sidx], y2tB[g][sidx]
                # group batches by which staging tile holds their rows
                groups = []
                for b in range(b0, b0 + NB):
                    r_lo = b * C + c
                    if r_lo + w <= 512:
                        kind = "A"
                    elif r_lo >= 512:
                        kind = "B"
                    else:
                        kind = "S"
                    if groups and groups[-1][0] == kind and kind != "S":
                        groups[-1][2] = b + 1
                    else:
                        groups.append([kind, b, b + 1])
                for kind, bl, bh in groups:
                    nb2 = bh - bl
                    if kind == "A":
                        nc.sync.dma_start(
                            kts[k][off:off + w, bl - b0:bh - b0, :],
                            tA.opt()[bl * C:bh * C, :]
                            .rearrange("(b c) n -> c b n", b=nb2)[c:c + w])
                    elif kind == "B":
                        nc.sync.dma_start(
                            kts[k][off:off + w, bl - b0:bh - b0, :],
                            tB.opt()[bl * C - 512:bh * C - 512, :]
                            .rearrange("(b c) n -> c b n", b=nb2)[c:c + w])
                    else:
                        cs = 512 - bl * C   # c < cs < c+w
                        nc.sync.dma_start(
                            kts[k][off:off + cs - c, bl - b0, :],
                            tA.opt()[bl * C + c:bl * C + cs, :])
                        nc.sync.dma_start(
                            kts[k][off + cs - c:off + w, bl - b0, :],
                            tB.opt()[0:c + w - cs, :])
            return kts

        def gate_mm(pi):
            b0 = NB * pi
            kts = load_ktiles(pi, d["xsT_own"].ap(), 0)
            zr_ps = [psum.tile([2 * H, NOWN], F32, name=f"zrps{b2}",
                               tag="acc", bufs=8) for b2 in range(NB)]
            for k in range(3):
                for b2 in range(NB):
                    nc.tensor.matmul(zr_ps[b2][:], wg_t[:, k, :],
                                     kts[k][:, b2, :],
                                     start=(k == 0), stop=(k == 2))
            zr4 = sb_sm.tile([2 * H, NB, NOWN], BF16, name="zr", tag="zr",
                             bufs=2)
            for b2 in range(NB):
                nc.scalar.activation(zr4[:, b2, :], zr_ps[b2][:],
                                     AF.Sigmoid, bias=bg_t[:])
            nc.scalar.dma_start(
                rt_dram.opt()[b0:b0 + NB].rearrange("b (h n) -> h b n",
                                                    h=H),
                zr4[H:2 * H, :, :])
            # kts[0][0:C] holds the direct input rows [state(64); x(2)]
            cT4 = sb_sm.tile([C, NB, NOWN], BF16, name="cT", tag="cT",
                             bufs=2)
            nc.vector.tensor_mul(cT4[0:H, :, :], zr4[0:H, :, :],
                                 kts[0][0:H, :, :])
            nc.vector.tensor_copy(cT4[H:C, :, :], kts[0][H:C, :, :])
            nc.scalar.dma_start(
                candT_dram.opt()[b0 * C:(b0 + NB) * C, :]
                .rearrange("(b c) n -> c b n", b=NB), cT4[:])
            return cT4

        def cand_stage(pi, cT4):
            # node-major candidate chunk pi (batch-aligned, 264 cols)
            for t in range(NT):
                ct4 = sb_sm.tile([128, NB, C], BF16, name="ctnm",
                                 tag="ctnm", bufs=2)
                for b2 in range(NB):
                    tp = psum.tile([128, C], BF16, name="ctps", tag="acc",
                                   bufs=8)
                    nc.tensor.transpose(
                        tp[:], cT4[:, b2, t * 128:(t + 1) * 128],
                        identb[0:C, 0:C])
                    nc.vector.tensor_copy(ct4[:, b2, :], tp[:])
                nc.scalar.dma_start(
                    candC_own(pi, t),
                    ct4[:].rearrange("p b c -> p (b c)"))
            allgather(candC[pi], candCo[pi])

        def update_pi(pi):
            b0 = NB * pi
            kts = load_ktiles(pi, candT_dram.opt(), 1)
            hc_ps = [psum.tile([H, NOWN], F32, name=f"hcps{b2}", tag="acc",
                               bufs=8) for b2 in range(NB)]
            for k in range(3):
                for b2 in range(NB):
                    nc.tensor.matmul(hc_ps[b2][:], wu_t[:, k, :],
                                     kts[k][:, b2, :],
                                     start=(k == 0), stop=(k == 2))
            hc4 = sb_sm.tile([H, NB, NOWN], BF16, name="hc", tag="zr",
                             bufs=2)
            for b2 in range(NB):
                nc.scalar.activation(hc4[:, b2, :], hc_ps[b2][:], AF.Tanh,
                                     bias=bu_t[:])
            # out = hc + r * (state - hc); state rows are xsT_own[0:H]
            stT4 = sb_sm.tile([H, NB, NOWN], BF16, name="stTu", tag="stg",
                              bufs=2)
            nc.sync.dma_start(
                stT4[:],
                d["xsT_own"].ap()[b0 * C:(b0 + NB) * C, :]
                .rearrange("(b c) n -> c b n", b=NB)[0:H])
            rT4 = sb_sm.tile([H, NB, NOWN], BF16, name="rT", tag="rT",
                             bufs=2)
            nc.sync.dma_start(
                rT4[:],
                rt_dram.opt()[b0:b0 + NB].rearrange("b (h n) -> h b n",
                                                    h=H))
            tmp4 = sb_sm.tile([H, NB, NOWN], BF16, name="tmp", tag="tmp",
                              bufs=2)
            nc.vector.tensor_sub(tmp4[:], stT4[:], hc4[:])
            nc.vector.tensor_mul(tmp4[:], rT4[:], tmp4[:])
            ot4 = sb_sm.tile([H, NB, NOWN], F32, name="ot", tag="ot",
                             bufs=1)
            nc.vector.tensor_add(ot4[:], hc4[:], tmp4[:])
            nc.scalar.dma_start(
                d["outT"].ap()[b0:b0 + NB].rearrange("b h n -> h b n"),
                ot4[:])

        # ======================= GCN 1 (gate) =======================
        def g1_load_mov(t4, q, hh):
            src = d["xs_main"].ap().rearrange("(q t p) f -> q p t f",
                                              p=128, t=NT)
            nc.scalar.dma_start(t4[:], src[q, :, :, hh * 512:(hh + 1) * 512])

        def g1_load_rag(t4, q):
            src = d["xs_rag"].ap().rearrange("(q t) p f -> q p t f", t=NT)
            nc.scalar.dma_start(t4[:], src[q])

        dyt1 = emit_hop1_pair("g1h1", 0, g1_load_mov, g1_load_rag,
                              y1t[0], True)
        dyt1()
        emit_hop2_pass("g1s0h2A", 0, 0, 0, y2tA[0][0], y2tB[0][0])
        emit_hop2_pass("g1s1h2A", 1, 0, 0, y2tA[0][1], y2tB[0][1])
        load_w_consts()
        ct0 = gate_mm(0)
        emit_hop2_pass("g1s0h2B", 0, 0, 1, y2tA[0][0], y2tB[0][0])
        cand_stage(0, ct0)
        emit_hop2_pass("g1s1h2B", 1, 0, 1, y2tA[0][1], y2tB[0][1])
        ct1 = gate_mm(1)
        cand_stage(1, ct1)
        ct2 = gate_mm(2)
        ct3 = gate_mm(3)
        cand_stage(2, ct2)
        cand_stage(3, ct3)

        # ======================= GCN 2 (update) =======================
        def g2_load_mov(t4, q, hh):
            if hh == 0:
                nc.sync.dma_start(t4[:, :, 0:CC4], candCo_q(0, q))
                nc.sync.dma_start(t4[:, :, CC4:512],
                                  candCo_q(1, q)[:, :, 0:512 - CC4])
            else:
                nc.sync.dma_start(t4[:, :, 0:2 * CC4 - 512],
                                  candCo_q(1, q)[:, :, 512 - CC4:CC4])
                nc.sync.dma_start(t4[:, :, 2 * CC4 - 512:3 * CC4 - 512],
                                  candCo_q(2, q))
                nc.sync.dma_start(t4[:, :, 3 * CC4 - 512:512],
                                  candCo_q(3, q)[:, :, 0:1024 - 3 * CC4])

        def g2_load_rag(t4, q):
            nc.sync.dma_start(t4[:], candCo_q(3, q)[:, :, 1024 - 3 * CC4:])

        dyt2 = emit_hop1_pair("g2h1", 1, g2_load_mov, g2_load_rag,
                              y1t[1], False)
        dyt2()
        emit_hop2_pass("g2s0h2A", 0, 1, 0, y2tA[1][0], y2tB[1][0])
        emit_hop2_pass("g2s1h2A", 1, 1, 0, y2tA[1][1], y2tB[1][1])
        update_pi(0)
        emit_hop2_pass("g2s0h2B", 0, 1, 1, y2tA[1][0], y2tB[1][0])
        emit_hop2_pass("g2s1h2B", 1, 1, 1, y2tA[1][1], y2tB[1][1])
        update_pi(1)
        update_pi(2)
        update_pi(3)


def prepare_in_maps(x, state, support0, support1, W_gate, b_gate,
                    W_update, b_update):
    BFNP = mybir.dt.np(BF16)
    xs = np.concatenate([x, state], axis=-1)          # [B, N, C]
    xs_nm = np.ascontiguousarray(
        xs.transpose(1, 0, 2).reshape(N, BC)).astype(BFNP)
    # feature-major input for W / elementwise uses [state(64); x(2)] rows
    sx_nm = np.ascontiguousarray(
        np.concatenate([state, x], axis=-1)
        .transpose(1, 0, 2).reshape(N, BC)).astype(np.float32)
    perm = np.r_[DIN:C, 0:DIN]                 # [x, state] -> [state, x]

    # fold the Chebyshev combination x2 = 2*A@x1 - x0 into W:
    # W0 -= (W2 + W4); W2 *= 2; W4 *= 2  (per 66-row block)
    def fold(W):
        Wf = np.ascontiguousarray(W, dtype=np.float32).copy()
        Wf[0:C] -= Wf[2 * C:3 * C] + Wf[4 * C:5 * C]
        Wf[2 * C:3 * C] *= 2.0
        Wf[4 * C:5 * C] *= 2.0
        return Wf

    Wg_dev = fold(W_gate)
    Wg_dev[0:C] = Wg_dev[0:C][perm]            # only the X-block reads xsT
    Wu_dev = fold(W_update)
    for j in range(5):                         # all of cand's blocks permute
        Wu_dev[j * C:(j + 1) * C] = Wu_dev[j * C:(j + 1) * C][perm]
    Wg_dev = Wg_dev.astype(BFNP)
    Wu_dev = Wu_dev.astype(BFNP)

    xs_main = np.ascontiguousarray(xs_nm[:, :MAIN])
    xs_rag = np.ascontiguousarray(xs_nm[:, MAIN:]).reshape(MT, 128, RAG)
    bg = np.ascontiguousarray(b_gate, dtype=np.float32).reshape(2 * H, 1)
    bu = np.ascontiguousarray(b_update, dtype=np.float32).reshape(H, 1)
    s0b = np.asarray(support0, dtype=np.float32).astype(BFNP)
    s1b = np.asarray(support1, dtype=np.float32).astype(BFNP)

    in_maps = []
    for r in range(NCORES):
        n0 = r * NOWN
        in_maps.append({
            "Ts": np.ascontiguousarray(
                np.stack([s0b[n0:n0 + NOWN, :].T,
                          s1b[n0:n0 + NOWN, :].T])),
            "xs_main": xs_main,
            "xs_rag": xs_rag,
            "xsT_own": np.ascontiguousarray(
                sx_nm[n0:n0 + NOWN].T).astype(BFNP),
            "Wg": Wg_dev, "bg": bg, "Wu": Wu_dev, "bu": bu,
        })
    return in_maps


def assemble_output(results):
    out = np.empty((B, N, H), dtype=np.float32)
    for r in range(NCORES):
        n0 = r * NOWN
        out[:, n0:n0 + NOWN, :] = results[r]["outT"].transpose(0, 2, 1)
    return out


def get_nc():
    if "nc" not in _NC_CACHE:
        _NC_CACHE["nc"] = build_nc()
    return _NC_CACHE["nc"]


def kernel(x, state, support0, support1, W_gate, b_gate, W_update, b_update):
    nc = get_nc()
    in_maps = prepare_in_maps(x, state, support0, support1,
                              W_gate, b_gate, W_update, b_update)
    prev = os.environ.get("BASS_NEVER_TRACE")
    os.environ["BASS_NEVER_TRACE"] = "1"
    try:
        res = run_bass_kernel_spmd(nc, in_maps, list(range(NCORES)),
                                   trace=False)
    finally:
        if prev is None:
            os.environ.pop("BASS_NEVER_TRACE", None)
        else:
            os.environ["BASS_NEVER_TRACE"] = prev
    return assemble_output(res.results)
